# revision 1
# baseline (speedup 1.0000x reference)
"""Trainium2 Bass kernel for nn_Attention_15109694947883.

Causal self-attention where (due to the reference's source quirk) q, k, v
all come from the first third of the qkv projection, so only
w_qkv[:, :1024] participates.

Sharding: head-parallel across 8 cores. Core c handles heads (2c, 2c+1)
for both batches: it gets w_qkv columns [128c, 128c+128) and w_out rows
[128c, 128c+128), computes its partial output [4096, 1024]; the host sums
the 8 partials and adds the bias.

v3 design (vs the 341us baseline; measured 243us via R=16385 For_i
amplification, TimelineSim 157us single-core):
  - bf16 everywhere on the matmul path (x, w1, w2, Q, scores-P, y) --
    halves HBM traffic; bf16 transposes run 1 cycle/row.
  - host-packed input layouts so the x load is 4 large fully-contiguous
    DMAs (16KB/descriptor) instead of 64 small ones: each DMA costs
    ~625ns of serialized HWDGE dispatch regardless of size.
  - softmax denominator folded into the PV matmul: lhsT is the 65-wide
    [V-dims | ones] slice of QNX, so out rows 0:64 = PV and row 64 = the
    denominator. Kills 160 separate denominator matmuls (~29us of PE).
  - causal diag-block masking moved off DVE: exp runs unmasked, then the
    idle Pool engine zeroes the upper-triangular part of the diagonal
    P^T block (affine_select on bf16 in SBUF).
  - exp activations widened to 1024 columns (two PSUM banks per
    activation) to halve ACT instruction overhead.
  - y written as [128,1024] bf16 rows: 32 stores of 2KB/partition.
  - emission schedule (sched=1): batch-0 scores start after half the
    projection (hides the x-load DMA), and each PV group is followed by
    a slice of the other batch's scores so the PE stays busy while DVE
    runs the normalize chain. Sim slightly prefers sched=0, but HW is
    ~29us faster with sched=1 (PE density keeps the HAM clock warm).
  - PE warmup burst during the first x-load DMA (HAM clock-gate stays
    at 2.4GHz), head-interleaved score windows (adjacent matmuls on
    different PE row tiles), and a software-pipelined batch-1 tail
    (pv_mm of group g+1 covers group g's DVE normalize-chain latency).
"""

import numpy as np

# Problem constants (hardcoded per harness contract)
B = 2
SEQ = 2048
DIM = 1024
HEADS = 16
DH = 64
SCALE = DH ** -0.5
N_CORES = 8
HD = 128          # head dims per core = 2 heads x 64
CB = 128          # key block
RB = 512          # row group block


def _split_waits(nc, mybir, maxw=1):
    """This walrus build rejects >maxw sync waits on one instruction
    (seen on Tile's tail drain). Split excess waits onto preceding
    same-engine NoOps — engines execute their stream in order, so the
    blocking semantics are identical."""
    n = 0
    for f in nc.m.functions:
        for bb in f.blocks:
            insts = list(bb.instructions)
            out = []
            for inst in insts:
                si = inst.sync_info
                if si is not None and si.on_wait and len(si.on_wait) > maxw:
                    waits = list(si.on_wait)
                    head, rest = waits[:-maxw], waits[-maxw:]
                    while head:
                        chunk, head = head[:maxw], head[maxw:]
                        nop = mybir.InstNoOp(
                            name=f"I-waitsplit-{nc.next_id()}", ins=[], outs=[]
                        )
                        nop.engine = inst.engine
                        nop.sync_info = mybir.SyncInfo(
                            on_wait=chunk, on_update=[]
                        )
                        out.append(nop)
                        n += 1
                    si.on_wait = rest
                out.append(inst)
            if len(out) != len(insts):
                bb.instructions = out
    return n


def build_nc(seq=SEQ, dim=DIM, b=B, loop_r=0, act_w=512, sp_bufs=2,
             yps_tag="yps", yps_bufs=2, ab_bufs=2, null_body=False, sched=1,
             hil=1, warm=20, tail2=1, pipe0=1):
    from contextlib import ExitStack

    import concourse.bass as bass
    import concourse.mybir as mybir
    import concourse.tile as tile
    from concourse.masks import make_identity

    f32 = mybir.dt.float32
    bf16 = mybir.dt.bfloat16

    nb = b * seq              # 4096 total rows
    kt = dim // 128           # 8 contraction tiles
    nsb = nb // 1024          # 4 projection super-blocks
    nblk = nb // 128          # 32 transpose blocks
    jcs = seq // CB           # 16 key blocks per batch
    gs = seq // RB            # 4 row groups per batch
    QB = 130                  # QNX cols per 128-row block: [h0 64|1|h1 64|1]

    nc = bass.Bass("TRN2", target_bir_lowering=False, debug=False)
    # host-packed x: xp[p, sb*8192 + k*1024 + c] = x[sb*1024+c, k*128+p]
    xp = nc.dram_tensor("xp", [128, kt * nb], bf16, kind="ExternalInput").ap()
    # host-packed w1: w1p[p, k*128 + j] = w_qkv[k*128+p, 128*core + j]
    w1p = nc.dram_tensor("w1p", [128, dim], bf16, kind="ExternalInput").ap()
    w2p = nc.dram_tensor("w2p", [HD, dim], bf16, kind="ExternalInput").ap()
    y = nc.dram_tensor("y", [nb, dim], bf16, kind="ExternalOutput").ap()
    itc = (nc.dram_tensor("itc", [1, 1], f32, kind="ExternalOutput").ap()
           if loop_r > 0 else None)

    mm = nc.tensor.matmul

    with tile.TileContext(nc) as tc, ExitStack() as ctx:
        cpool = ctx.enter_context(tc.tile_pool(name="consts", bufs=1))
        ident = cpool.tile([128, 128], bf16, tag="ident")
        make_identity(nc, ident[:])

        wpool = ctx.enter_context(tc.tile_pool(name="w", bufs=1))
        W1 = wpool.tile([128, dim], bf16, tag="w1")
        nc.sync.dma_start(W1[:], w1p[:, :])
        W2 = wpool.tile([128, dim], bf16, tag="w2")
        nc.sync.dma_start(W2[:], w2p[:, :])

        qpool = ctx.enter_context(tc.tile_pool(name="q", bufs=1))
        QT = qpool.tile([128, nb], bf16, tag="qt")       # [head-dim, row]
        QNX = qpool.tile([128, nblk * QB], bf16, tag="qnx")

        psum = ctx.enter_context(tc.tile_pool(name="ps", bufs=1, space="PSUM"))
        ptpool = ctx.enter_context(tc.tile_pool(name="pt", bufs=1))
        spool = ctx.enter_context(tc.tile_pool(name="sm", bufs=2))
        onpool = ctx.enter_context(tc.tile_pool(name="on", bufs=2))
        ypool = ctx.enter_context(tc.tile_pool(name="ysb", bufs=3))
        xpool = ctx.enter_context(tc.tile_pool(name="xt", bufs=2))

        if loop_r > 0:
            itile = cpool.tile([1, 1], f32, tag="itile")
            nc.gpsimd.memset(itile[:], 0.0)

        loop_ctx = ExitStack()
        if loop_r > 0:
            loop_ctx.enter_context(tc.For_i(0, loop_r, 1))
            it2 = cpool.tile([1, 1], f32, tag="it2")
            nc.scalar.add(it2[:], itile[:], 1.0)
            nc.vector.tensor_copy(itile[:], it2[:])

        if null_body:
            # minimal loop body: one tiny matmul + one y-store, to measure
            # the fixed per-iteration For_i overhead
            nps = psum.tile([128, 128], f32, tag="sp", bufs=sp_bufs,
                            padded_shape=[128, act_w])
            mm(nps[:], W1[:, 0:128], W2[:, 0:128],
               start=True, stop=True)
            nsb_t = ypool.tile([128, 128], bf16, tag="ysb",
                               padded_shape=[128, dim])
            nc.vector.tensor_copy(nsb_t[:], nps[:])
            nc.sync.dma_start(y[0:128, 0:128], nsb_t[:])
            loop_ctx.close()
            if loop_r > 0:
                nc.sync.dma_start(itc[:], itile[:])
            return nc

        # ---- Phase 1: projection QT = w1^T x^T, transposes into QNX ----
        # QNX per block: [h0 dims 0:64 | ones | h1 dims 0:64 | ones];
        # memset(1.0) first, copies overwrite the data columns.
        nc.gpsimd.memset(QNX[:], 1.0)
        if warm:
            # keep the PE HAM clock-gate warm while the first x tiles stream
            # in: dummy accumulating matmuls on already-resident weights
            wps = psum.tile([128, RB], f32, tag="pa", bufs=ab_bufs)
            for i in range(warm):
                mm(wps[0:128, :], W1[:, (i % 2) * 128:(i % 2) * 128 + 128],
                   W2[:, 0:RB], start=(i == 0), stop=(i == warm - 1),
                   skip_group_check=True)

        def emit_p1(sb):
            xsb = xpool.tile([128, kt * 1024], bf16, tag="xsb", bufs=2)
            nc.sync.dma_start(xsb[:], xp[:, sb * kt * 1024:(sb + 1) * kt * 1024])
            qhs = []
            for half in range(2):
                qps = psum.tile([128, act_w], f32, tag="sp", bufs=sp_bufs,
                                padded_shape=[128, act_w])
                qhs.append(qps)
                for k in range(kt):
                    mm(qps[:, 0:512],
                       W1[:, k * 128:(k + 1) * 128],
                       xsb[:, k * 1024 + half * 512:k * 1024 + (half + 1) * 512],
                       start=(k == 0), stop=(k == kt - 1),
                       skip_group_check=True)
            for half in range(2):
                nc.vector.tensor_copy(
                    QT[:, sb * 1024 + half * 512:sb * 1024 + (half + 1) * 512],
                    qhs[half][:, 0:512])
            for t in range(8):
                col = sb * 1024 + t * 128
                blk = col // 128
                tps = psum.tile([128, 128], bf16, tag="pb", bufs=ab_bufs,
                                padded_shape=[128, 2 * RB])
                nc.tensor.transpose(tps[:], QT[:, col:col + 128], ident[:])
                nc.vector.tensor_copy(QNX[:, blk * QB:blk * QB + 64],
                                      tps[:, 0:64])
                nc.vector.tensor_copy(QNX[:, blk * QB + 65:blk * QB + 129],
                                      tps[:, 64:128])

        # ---- Phase 2: attention ----
        PTs = [dict() for _ in range(b)]

        def emit_scores(bi, jc_lo, jc_hi):
            base = bi * seq
            PT = PTs[bi]
            for jc in range(jc_lo, jc_hi):
                r0 = CB * jc
                cw = seq - r0
                for h in range(2):
                    pt = ptpool.tile([128, cw], bf16, tag=f"pt{h}_{jc}",
                                     bufs=2)
                    PT[(h, jc)] = pt
                if hil:
                    order = [(h, t) for t in range(r0 // act_w, seq // act_w)
                             for h in range(2)]
                else:
                    order = [(h, t) for h in range(2)
                             for t in range(r0 // act_w, seq // act_w)]
                for h, t in order:
                    pt = PT[(h, jc)]
                    lhsT = QT[64 * h:64 * h + 64, base + r0:base + r0 + 128]
                    ws = max(act_w * t, r0)
                    we = act_w * (t + 1)
                    sp = psum.tile([128, act_w], f32, tag="sp",
                                   bufs=sp_bufs, padded_shape=[128, act_w])
                    cs = ws
                    while cs < we:
                        ce = min(we, (cs // 512 + 1) * 512)
                        mm(sp[:, cs - act_w * t:ce - act_w * t], lhsT,
                           QT[64 * h:64 * h + 64, base + cs:base + ce],
                           start=True, stop=True,
                           tile_position=(64 * h, 0))
                        cs = ce
                    nc.scalar.activation(
                        pt[:, ws - r0:we - r0],
                        sp[:, ws - act_w * t:we - act_w * t],
                        mybir.ActivationFunctionType.Exp,
                        bias=0.0, scale=float(SCALE))
                for h in range(2):
                    # zero the strictly-upper part of the diagonal block
                    # (query col rr < key row c) on the idle Pool engine
                    pt = PT[(h, jc)]
                    nc.gpsimd.affine_select(
                        out=pt[:, 0:128], in_=pt[:, 0:128],
                        compare_op=mybir.AluOpType.is_ge, fill=0.0,
                        base=0, pattern=[[1, 128]], channel_multiplier=-1,
                    )

        def emit_pv_mm(bi, g):
            """PV+denominator matmuls for row group g; returns the 'on' tile
            (normalized O^T) whose out-projection the caller emits later."""
            base = bi * seq
            PT = PTs[bi]
            A = psum.tile([128, RB], f32, tag="pa", bufs=ab_bufs)
            Bp = psum.tile([128, RB], f32, tag="pb", bufs=ab_bufs,
                           padded_shape=[128, RB])
            njc = (g + 1) * (RB // CB)
            for jc in range(njc):
                r0 = CB * jc
                cs = max(RB * g, r0)
                w = RB * (g + 1) - cs
                blk = bi * jcs + jc
                for h, T in ((0, A), (1, Bp)):
                    pts = PT[(h, jc)][:, cs - r0:cs - r0 + w]
                    mm(T[0:65, cs - RB * g:cs - RB * g + w],
                       QNX[:, blk * QB + 65 * h:blk * QB + 65 * h + 65],
                       pts,
                       start=(jc == 0), stop=(jc == njc - 1),
                       skip_group_check=True)
            # reciprocal of the folded denominators (partition 64)
            rr = spool.tile([128, 1024], bf16, tag="rr")
            with nc.allow_low_precision(reason="1/den in bf16: 0.4% rel"):
                nc.vector.reciprocal(rr[64:65, 0:RB], A[64:65, :])
                nc.vector.reciprocal(rr[64:65, RB:2 * RB], Bp[64:65, :])
            # broadcast partition 64 -> partitions 0:64 (h0 cols 0:512,
            # h1 cols 512:1024)
            bc = spool.tile([128, 1024], bf16, tag="bc")
            nc.sync.dma_start(
                bc[0:64, :],
                rr[64:65, :].unsqueeze(1).to_broadcast([1, 64, 1024]))
            on = onpool.tile([128, RB], bf16, tag="on")
            tmp = onpool.tile([128, RB], bf16, tag="tmp")
            nc.vector.tensor_mul(on[0:64, :], A[0:64, :], bc[0:64, 0:RB])
            nc.vector.tensor_mul(tmp[0:64, :], Bp[0:64, :],
                                 bc[0:64, RB:2 * RB])
            # partition shift h1 dims into on[64:128] (SBUF->SBUF DMA)
            nc.sync.dma_start(on[64:128, :], tmp[0:64, :])
            return on

        def emit_outproj(bi, g, on):
            base = bi * seq
            for rb_i in range(RB // 128):
                ysb = ypool.tile([128, dim], bf16, tag="ysb")
                for eb in range(2):
                    yps = psum.tile([128, 512], f32, tag=yps_tag,
                                    bufs=yps_bufs)
                    mm(yps[:], on[:, rb_i * 128:(rb_i + 1) * 128],
                       W2[:, eb * 512:(eb + 1) * 512],
                       start=True, stop=True)
                    nc.vector.tensor_copy(ysb[:, eb * 512:(eb + 1) * 512],
                                          yps[:])
                r_out = base + RB * g + 128 * rb_i
                nc.sync.dma_start(y[r_out:r_out + 128, :], ysb[:])

        # Schedule: batch-0 scores start as soon as batch-0's projection
        # (sb 0-1) lands, hiding the sb 2-3 x-loads; PV groups interleave
        # with the other batch's scores so PE stays busy while DVE runs the
        # normalize chains.
        if sched == 1:
            emit_p1(0)
            emit_p1(1)
            emit_scores(0, 0, 6)
            emit_p1(2)
            emit_scores(0, 6, 11)
            emit_p1(3)
            emit_scores(0, 11, jcs)
            sc1 = [(0, 2), (2, 7), (7, 12), (12, jcs)]
            if pipe0:
                # pipeline batch-0 PV one group ahead as well: the next
                # group's PV matmuls + the scores slice cover the DVE
                # normalize chain before each out-projection
                pend = emit_pv_mm(0, 0)
                emit_scores(1, *sc1[0])
                for g in range(1, gs):
                    nxt = emit_pv_mm(0, g)
                    emit_outproj(0, g - 1, pend)
                    pend = nxt
                    emit_scores(1, *sc1[g])
                emit_outproj(0, gs - 1, pend)
            else:
                for g in range(gs):
                    pend = emit_pv_mm(0, g)
                    emit_scores(1, *sc1[g])
                    emit_outproj(0, g, pend)
            if tail2:
                # software-pipeline the scores-free batch-1 tail: pv_mm of
                # the next group covers the previous group's DVE normalize
                # chain latency before its out-projection
                pend = emit_pv_mm(1, 0)
                for g in range(1, gs):
                    nxt = emit_pv_mm(1, g)
                    emit_outproj(1, g - 1, pend)
                    pend = nxt
                emit_outproj(1, gs - 1, pend)
            else:
                for g in range(gs):
                    pend = emit_pv_mm(1, g)
                    emit_outproj(1, g, pend)
        else:
            for sb in range(nsb):
                emit_p1(sb)
            emit_scores(0, 0, jcs)
            emit_scores(1, 0, 4)
            for g in range(gs):
                pend = emit_pv_mm(0, g)
                emit_outproj(0, g, pend)
            emit_scores(1, 4, jcs)
            for g in range(gs):
                pend = emit_pv_mm(1, g)
                emit_outproj(1, g, pend)

        loop_ctx.close()
        if loop_r > 0:
            nc.sync.dma_start(itc[:], itile[:])

    return nc


# ---------------------------------------------------------------------------
# Host wrapper
# ---------------------------------------------------------------------------

_CACHE = {}


def _get_nc():
    if "nc" not in _CACHE:
        import concourse.mybir as mybir
        nc = build_nc()
        _split_waits(nc, mybir, maxw=1)
        _CACHE["nc"] = nc
    return _CACHE["nc"]


def _bf16(a):
    import ml_dtypes
    return np.ascontiguousarray(a.astype(ml_dtypes.bfloat16))


def build_in_maps(x, w_qkv, w_out):
    """Pack host inputs into the per-core DMA-friendly layouts."""
    xf = np.asarray(x, np.float32).reshape(B * SEQ, DIM)
    # xp[p, sb*8192 + k*1024 + c] = xf[sb*1024 + c, k*128 + p]
    xp = xf.reshape(4, 1024, 8, 128).transpose(3, 0, 2, 1).reshape(128, -1)
    xp = _bf16(xp)
    w_qkv = np.asarray(w_qkv, np.float32)
    w_out = np.asarray(w_out, np.float32)
    in_maps = []
    for c in range(N_CORES):
        w1c = w_qkv[:, HD * c:HD * (c + 1)]          # [1024, 128]
        w1p = w1c.reshape(8, 128, 128).transpose(1, 0, 2).reshape(128, 1024)
        in_maps.append({
            "xp": xp,
            "w1p": _bf16(w1p),
            "w2p": _bf16(w_out[HD * c:HD * (c + 1), :]),
        })
    return in_maps


def kernel(x, w_qkv, w_out, b_out):
    import jax
    jax.devices()  # ensure axon backend initialized
    from concourse.bass_utils import run_bass_kernel_spmd

    nc = _get_nc()
    in_maps = build_in_maps(x, w_qkv, w_out)
    res = run_bass_kernel_spmd(nc, in_maps, list(range(N_CORES)))
    acc = np.zeros((B * SEQ, DIM), dtype=np.float32)
    for c in range(N_CORES):
        acc += np.asarray(res.results[c]["y"], dtype=np.float32)
    acc += np.asarray(b_out, dtype=np.float32)[None, :]
    return acc.reshape(B, SEQ, DIM)



# revision 54
# speedup vs baseline: 1.5795x; 1.5795x over previous
"""Trainium2 Bass kernel for nn_Attention_15109694947883.

Causal self-attention where (due to the reference's source quirk) q, k, v
all come from the first third of the qkv projection, so only
w_qkv[:, :1024] participates.

Sharding: head-parallel across 8 cores. Core c handles heads (2c, 2c+1)
for both batches: it gets w_qkv columns [128c, 128c+128) and w_out rows
[128c, 128c+128), computes its partial output [4096, 1024]; the host sums
the 8 partials and adds the bias.

v4 changes (TimelineSim 157us -> 131.8us single-core; HW per-iter
243us -> ~235us and below, R=16385 For_i amplification):
  - projection PSUM moved off the scores "sp" tag onto "pa" (the big
    one: -18us sim — proj and scores no longer serialize on the same
    PSUM rotation).
  - exp activations at act_w=1024 (two PSUM banks per exp): fewer,
    wider ACT instructions; out-proj yps shares the freed "pa" tag.
  - merged QNX copy: one strided DVE copy per transpose block
    ([2x64-col groups at stride 65]) instead of two.
  - normalize: mulB emitted before the partition-shift DMA so the
    shift overlaps mulA; optional Pool-engine shift (shift_eng).
  - final out-proj group stores y per 512-col half (ysplit) to pull
    the last DMA earlier; warm0 pre-warms PE on the identity before
    W1 lands.
  - sched=2/3: batch-1 score windows woven one unit at a time between
    batch-0 PV/out-proj matmuls (PE density for the HW HAM clock).
  - NOTE hard-learned HW constraints: Pool (gpsimd) cannot read PSUM
    (compile error); matmul out must be fp32 PSUM at base partition
    0/32/64; stream_shuffle is quadrant-local.

v3 design (vs the 341us baseline; measured 243us via R=16385 For_i
amplification, TimelineSim 157us single-core):
  - bf16 everywhere on the matmul path (x, w1, w2, Q, scores-P, y) --
    halves HBM traffic; bf16 transposes run 1 cycle/row.
  - host-packed input layouts so the x load is 4 large fully-contiguous
    DMAs (16KB/descriptor) instead of 64 small ones: each DMA costs
    ~625ns of serialized HWDGE dispatch regardless of size.
  - softmax denominator folded into the PV matmul: lhsT is the 65-wide
    [V-dims | ones] slice of QNX, so out rows 0:64 = PV and row 64 = the
    denominator. Kills 160 separate denominator matmuls (~29us of PE).
  - causal diag-block masking moved off DVE: exp runs unmasked, then the
    idle Pool engine zeroes the upper-triangular part of the diagonal
    P^T block (affine_select on bf16 in SBUF).
  - exp activations widened to 1024 columns (two PSUM banks per
    activation) to halve ACT instruction overhead.
  - y written as [128,1024] bf16 rows: 32 stores of 2KB/partition.
  - emission schedule (sched=1): batch-0 scores start after half the
    projection (hides the x-load DMA), and each PV group is followed by
    a slice of the other batch's scores so the PE stays busy while DVE
    runs the normalize chain. Sim slightly prefers sched=0, but HW is
    ~29us faster with sched=1 (PE density keeps the HAM clock warm).
  - PE warmup burst during the first x-load DMA (HAM clock-gate stays
    at 2.4GHz), head-interleaved score windows (adjacent matmuls on
    different PE row tiles), and a software-pipelined batch-1 tail
    (pv_mm of group g+1 covers group g's DVE normalize-chain latency).
"""

import numpy as np

# Problem constants (hardcoded per harness contract)
B = 2
SEQ = 2048
DIM = 1024
HEADS = 16
DH = 64
SCALE = DH ** -0.5
N_CORES = 8
HD = 128          # head dims per core = 2 heads x 64
CB = 128          # key block
RB = 512          # row group block


def _split_waits(nc, mybir, maxw=1):
    """This walrus build rejects >maxw sync waits on one instruction
    (seen on Tile's tail drain). Split excess waits onto preceding
    same-engine NoOps — engines execute their stream in order, so the
    blocking semantics are identical."""
    n = 0
    for f in nc.m.functions:
        for bb in f.blocks:
            insts = list(bb.instructions)
            out = []
            for inst in insts:
                si = inst.sync_info
                if si is not None and si.on_wait and len(si.on_wait) > maxw:
                    waits = list(si.on_wait)
                    head, rest = waits[:-maxw], waits[-maxw:]
                    while head:
                        chunk, head = head[:maxw], head[maxw:]
                        nop = mybir.InstNoOp(
                            name=f"I-waitsplit-{nc.next_id()}", ins=[], outs=[]
                        )
                        nop.engine = inst.engine
                        nop.sync_info = mybir.SyncInfo(
                            on_wait=chunk, on_update=[]
                        )
                        out.append(nop)
                        n += 1
                    si.on_wait = rest
                out.append(inst)
            if len(out) != len(insts):
                bb.instructions = out
    return n


def build_nc(seq=SEQ, dim=DIM, b=B, loop_r=0, act_w=1024, sp_bufs=2,
             yps_tag="pa", yps_bufs=2, ab_bufs=2, null_body=False, sched=2,
             hil=1, warm=10, tail2=1, pipe0=1, ysb_eng="dve", qnx_merge=1,
             yps_bf16=0, osplit=0, aff_early=0, qps_tag="pa", bcast=0,
             shift_eng="dma", tail_mix=0, xsplit=0, warm0=0, ysplit=0,
             sr=0):
    from contextlib import ExitStack

    import concourse.bass as bass
    import concourse.mybir as mybir
    import concourse.tile as tile
    from concourse.masks import make_identity

    f32 = mybir.dt.float32
    bf16 = mybir.dt.bfloat16

    nb = b * seq              # 4096 total rows
    kt = dim // 128           # 8 contraction tiles
    nsb = nb // 1024          # 4 projection super-blocks
    nblk = nb // 128          # 32 transpose blocks
    jcs = seq // CB           # 16 key blocks per batch
    gs = seq // RB            # 4 row groups per batch
    QB = 130                  # QNX cols per 128-row block: [h0 64|1|h1 64|1]

    nc = bass.Bass("TRN2", target_bir_lowering=False, debug=False)
    # host-packed x: xp[p, sb*8192 + k*1024 + c] = x[sb*1024+c, k*128+p]
    xp = nc.dram_tensor("xp", [128, kt * nb], bf16, kind="ExternalInput").ap()
    # host-packed w1: w1p[p, k*128 + j] = w_qkv[k*128+p, 128*core + j]
    w1p = nc.dram_tensor("w1p", [128, dim], bf16, kind="ExternalInput").ap()
    # osplit: w2p[p, h*dim + j] = w_out[HD*core + 64*h + p, j]  ([64, 2*dim])
    w2p = (nc.dram_tensor("w2p", [64, 2 * dim], bf16, kind="ExternalInput").ap()
           if osplit else
           nc.dram_tensor("w2p", [HD, dim], bf16, kind="ExternalInput").ap())
    y = nc.dram_tensor("y", [nb, dim], bf16, kind="ExternalOutput").ap()
    itc = (nc.dram_tensor("itc", [1, 1], f32, kind="ExternalOutput").ap()
           if loop_r > 0 else None)

    mm = nc.tensor.matmul

    with tile.TileContext(nc) as tc, ExitStack() as ctx:
        cpool = ctx.enter_context(tc.tile_pool(name="consts", bufs=1))
        ident = cpool.tile([128, 128], bf16, tag="ident")
        make_identity(nc, ident[:])

        wpool = ctx.enter_context(tc.tile_pool(name="w", bufs=1))
        W1 = wpool.tile([128, dim], bf16, tag="w1")
        nc.sync.dma_start(W1[:], w1p[:, :])
        if osplit:
            W2 = wpool.tile([64, 2 * dim], bf16, tag="w2")
        else:
            W2 = wpool.tile([128, dim], bf16, tag="w2")
        nc.sync.dma_start(W2[:], w2p[:, :])

        qpool = ctx.enter_context(tc.tile_pool(name="q", bufs=1))
        QT = qpool.tile([128, nb], bf16, tag="qt")       # [head-dim, row]
        QNX = qpool.tile([128, nblk * QB], bf16, tag="qnx")

        psum = ctx.enter_context(tc.tile_pool(name="ps", bufs=1, space="PSUM"))
        ptpool = ctx.enter_context(tc.tile_pool(name="pt", bufs=1))
        spool = ctx.enter_context(tc.tile_pool(name="sm", bufs=2))
        onpool = ctx.enter_context(tc.tile_pool(name="on", bufs=2))
        ypool = ctx.enter_context(tc.tile_pool(name="ysb",
                                               bufs=2 if osplit else 3))
        xpool = ctx.enter_context(tc.tile_pool(name="xt", bufs=2))

        if loop_r > 0:
            itile = cpool.tile([1, 1], f32, tag="itile")
            nc.gpsimd.memset(itile[:], 0.0)

        loop_ctx = ExitStack()
        if loop_r > 0:
            loop_ctx.enter_context(
                tc.For_i(0, loop_r, 1, staggered_reset=bool(sr)))
            it2 = cpool.tile([1, 1], f32, tag="it2")
            nc.scalar.add(it2[:], itile[:], 1.0)
            nc.vector.tensor_copy(itile[:], it2[:])

        if null_body:
            # minimal loop body: one tiny matmul + one y-store, to measure
            # the fixed per-iteration For_i overhead.
            # null_body=2: counter only (no mm/copy/DMA); 3: +mm+copy, no DMA
            if null_body != 2:
                nps = psum.tile([128, 128], f32, tag="sp", bufs=sp_bufs,
                                padded_shape=[128, act_w])
                mm(nps[:], W1[:, 0:128], W1[:, 0:128],
                   start=True, stop=True)
                nsb_t = ypool.tile([128, 128], bf16, tag="ysb",
                                   padded_shape=[128, dim])
                nc.vector.tensor_copy(nsb_t[:], nps[:])
                if null_body != 3:
                    nc.sync.dma_start(y[0:128, 0:128], nsb_t[:])
            loop_ctx.close()
            if loop_r > 0:
                nc.sync.dma_start(itc[:], itile[:])
            return nc

        # ---- Phase 1: projection QT = w1^T x^T, transposes into QNX ----
        # QNX per block: [h0 dims 0:64 | ones | h1 dims 0:64 | ones];
        # memset(1.0) first, copies overwrite the data columns.
        nc.gpsimd.memset(QNX[:], 1.0)
        if warm0:
            # pre-warm on the engine-generated identity (no DMA dependency):
            # keeps the PE HAM clock ramping before W1 even lands
            w0ps = psum.tile([128, 128], f32, tag="pa", bufs=ab_bufs,
                             padded_shape=[128, 512])
            for i in range(warm0):
                mm(w0ps[:], ident[:], ident[:],
                   start=(i == 0), stop=(i == warm0 - 1),
                   skip_group_check=True)
        if warm:
            # keep the PE HAM clock-gate warm while the first x tiles stream
            # in: dummy accumulating matmuls on already-resident weights
            wps = psum.tile([128, RB], f32, tag="pa", bufs=ab_bufs)
            wrhs = W1 if osplit else W2
            for i in range(warm):
                mm(wps[0:128, :], W1[:, (i % 2) * 128:(i % 2) * 128 + 128],
                   wrhs[:, 0:RB], start=(i == 0), stop=(i == warm - 1),
                   skip_group_check=True)

        def emit_p1(sb):
            xsb = xpool.tile([128, kt * 1024], bf16, tag="xsb", bufs=2)
            if xsplit:
                # two DMAs: c-halves of every k-tile, so the half-0 matmuls
                # can start as soon as the first (smaller) DMA lands
                xv = xsb[:].rearrange("p (k c) -> p k c", k=kt)
                sv = xp[:, sb * kt * 1024:(sb + 1) * kt * 1024].rearrange(
                    "p (k c) -> p k c", k=kt)
                nc.sync.dma_start(xv[:, :, 0:512], sv[:, :, 0:512])
                nc.sync.dma_start(xv[:, :, 512:1024], sv[:, :, 512:1024])
            else:
                nc.sync.dma_start(
                    xsb[:], xp[:, sb * kt * 1024:(sb + 1) * kt * 1024])
            qhs = []
            for half in range(2):
                qps = psum.tile([128, 512], f32,
                                tag=qps_tag,
                                bufs=(sp_bufs if qps_tag == "sp" else ab_bufs),
                                padded_shape=[128, max(512, act_w)]
                                if qps_tag == "sp" else None)
                qhs.append(qps)
                for k in range(kt):
                    mm(qps[:, 0:512],
                       W1[:, k * 128:(k + 1) * 128],
                       xsb[:, k * 1024 + half * 512:k * 1024 + (half + 1) * 512],
                       start=(k == 0), stop=(k == kt - 1),
                       skip_group_check=True)
            for half in range(2):
                nc.vector.tensor_copy(
                    QT[:, sb * 1024 + half * 512:sb * 1024 + (half + 1) * 512],
                    qhs[half][:, 0:512])
            for t in range(8):
                col = sb * 1024 + t * 128
                blk = col // 128
                tps = psum.tile([128, 128], bf16, tag="pb", bufs=ab_bufs,
                                padded_shape=[128, 2 * RB])
                nc.tensor.transpose(tps[:], QT[:, col:col + 128], ident[:])
                if qnx_merge:
                    # single strided copy: 2 groups of 64 cols, dst stride 65
                    dst = QNX[:, blk * QB:blk * QB + 130].rearrange(
                        "p (two c) -> p two c", two=2)[:, :, 0:64]
                    src = tps[:].rearrange("p (two c) -> p two c", two=2)
                    nc.vector.tensor_copy(dst, src)
                else:
                    nc.vector.tensor_copy(QNX[:, blk * QB:blk * QB + 64],
                                          tps[:, 0:64])
                    nc.vector.tensor_copy(QNX[:, blk * QB + 65:blk * QB + 129],
                                          tps[:, 64:128])

        # ---- Phase 2: attention ----
        PTs = [dict() for _ in range(b)]

        def emit_scores_gen(bi, jc_lo, jc_hi):
            """Generator form: yields after each (h, t) window unit so the
            caller can weave score emission between other PE work."""
            base = bi * seq
            PT = PTs[bi]
            for jc in range(jc_lo, jc_hi):
                r0 = CB * jc
                cw = seq - r0
                for h in range(2):
                    pt = ptpool.tile([128, cw], bf16, tag=f"pt{h}_{jc}",
                                     bufs=2)
                    PT[(h, jc)] = pt
                if hil:
                    order = [(h, t) for t in range(r0 // act_w, seq // act_w)
                             for h in range(2)]
                else:
                    order = [(h, t) for h in range(2)
                             for t in range(r0 // act_w, seq // act_w)]
                def emit_mask(h):
                    # zero the strictly-upper part of the diagonal block
                    # (query col rr < key row c) on the idle Pool engine
                    pt = PT[(h, jc)]
                    nc.gpsimd.affine_select(
                        out=pt[:, 0:128], in_=pt[:, 0:128],
                        compare_op=mybir.AluOpType.is_ge, fill=0.0,
                        base=0, pattern=[[1, 128]], channel_multiplier=-1,
                    )
                diag_t = r0 // act_w
                for h, t in order:
                    pt = PT[(h, jc)]
                    lhsT = QT[64 * h:64 * h + 64, base + r0:base + r0 + 128]
                    ws = max(act_w * t, r0)
                    we = act_w * (t + 1)
                    sp = psum.tile([128, act_w], f32, tag="sp",
                                   bufs=sp_bufs, padded_shape=[128, act_w])
                    cs = ws
                    while cs < we:
                        ce = min(we, (cs // 512 + 1) * 512)
                        mm(sp[:, cs - act_w * t:ce - act_w * t], lhsT,
                           QT[64 * h:64 * h + 64, base + cs:base + ce],
                           start=True, stop=True,
                           tile_position=(64 * h, 0))
                        cs = ce
                    nc.scalar.activation(
                        pt[:, ws - r0:we - r0],
                        sp[:, ws - act_w * t:we - act_w * t],
                        mybir.ActivationFunctionType.Exp,
                        bias=0.0, scale=float(SCALE))
                    if aff_early and t == diag_t:
                        emit_mask(h)
                    yield
                if not aff_early:
                    for h in range(2):
                        emit_mask(h)

        def emit_scores(bi, jc_lo, jc_hi):
            for _ in emit_scores_gen(bi, jc_lo, jc_hi):
                pass

        def make_pump(gen):
            def pump(n=1):
                for _ in range(n):
                    if next(gen, "end") == "end":
                        break
            return pump

        ENG_CPY = {"dve": nc.vector.tensor_copy,
                   "pool": nc.gpsimd.tensor_copy,
                   "act": nc.scalar.copy}
        ENG_MUL = {"dve": nc.vector.tensor_mul,
                   "pool": nc.gpsimd.tensor_mul}

        def emit_pv_mm(bi, g, mulb_eng="dve", pump=None):
            """PV+denominator matmuls for row group g; returns the 'on' tile
            (normalized O^T) whose out-projection the caller emits later."""
            base = bi * seq
            PT = PTs[bi]
            A = psum.tile([128, RB], f32, tag="pa", bufs=ab_bufs)
            Bp = psum.tile([128, RB], f32, tag="pb", bufs=ab_bufs,
                           padded_shape=[128, RB])
            njc = (g + 1) * (RB // CB)
            for jc in range(njc):
                r0 = CB * jc
                cs = max(RB * g, r0)
                w = RB * (g + 1) - cs
                blk = bi * jcs + jc
                for h, T in ((0, A), (1, Bp)):
                    pts = PT[(h, jc)][:, cs - r0:cs - r0 + w]
                    mm(T[0:65, cs - RB * g:cs - RB * g + w],
                       QNX[:, blk * QB + 65 * h:blk * QB + 65 * h + 65],
                       pts,
                       start=(jc == 0), stop=(jc == njc - 1),
                       skip_group_check=True)
                if pump is not None and jc % 2 == 1:
                    pump(1)
            # reciprocal of the folded denominators (partition 64)
            rr = spool.tile([128, 1024], bf16, tag="rr")
            with nc.allow_low_precision(reason="1/den in bf16: 0.4% rel"):
                nc.vector.reciprocal(rr[64:65, 0:RB], A[64:65, :])
                nc.vector.reciprocal(rr[64:65, RB:2 * RB], Bp[64:65, :])
            # broadcast partition 64 -> partitions 0:64 (h0 cols 0:512,
            # h1 cols 512:1024)
            bc = spool.tile([128, 1024], bf16, tag="bc")
            if bcast:
                nc.gpsimd.partition_broadcast(bc[0:64, :], rr[64:65, :])
            else:
                nc.sync.dma_start(
                    bc[0:64, :],
                    rr[64:65, :].unsqueeze(1).to_broadcast([1, 64, 1024]))
            if osplit:
                # keep both heads at partitions 0:64 ([64, 2*RB]: h0 cols
                # 0:RB, h1 cols RB:2RB); out-proj contracts each head
                # separately (no partition-shift DMA needed)
                on = onpool.tile([64, 2 * RB], bf16, tag="on")
                nc.vector.tensor_mul(on[0:64, 0:RB], A[0:64, :],
                                     bc[0:64, 0:RB])
                nc.vector.tensor_mul(on[0:64, RB:2 * RB], Bp[0:64, :],
                                     bc[0:64, RB:2 * RB])
                return on
            on = onpool.tile([128, RB], bf16, tag="on")
            tmp = onpool.tile([128, RB], bf16, tag="tmp")
            # mulB first so the partition shift overlaps mulA on DVE
            ENG_MUL[mulb_eng](tmp[0:64, :], Bp[0:64, :],
                              bc[0:64, RB:2 * RB])
            # partition shift h1 dims into on[64:128]
            if shift_eng == "dve":
                nc.vector.tensor_copy(on[64:128, :], tmp[0:64, :])
            elif shift_eng == "pool":
                nc.gpsimd.tensor_copy(on[64:128, :], tmp[0:64, :])
            else:
                nc.sync.dma_start(on[64:128, :], tmp[0:64, :])
            nc.vector.tensor_mul(on[0:64, :], A[0:64, :], bc[0:64, 0:RB])
            return on

        def emit_outproj(bi, g, on, cps=None, pump=None):
            base = bi * seq
            if cps is None:
                cps = [ysb_eng] * 8
            fine = ysplit and bi == b - 1 and g == gs - 1
            for rb_i in range(RB // 128):
                if pump is not None:
                    pump(1)
                ysb = ypool.tile([128, dim], bf16, tag="ysb")
                for eb in range(2):
                    cpy = ENG_CPY[cps[rb_i * 2 + eb]]
                    yps = psum.tile([128, 512], bf16 if yps_bf16 else f32,
                                    tag=yps_tag, bufs=yps_bufs,
                                    padded_shape=([128, act_w]
                                                  if yps_tag == "sp" else None))
                    if osplit:
                        mm(yps[:], on[0:64, rb_i * 128:(rb_i + 1) * 128],
                           W2[0:64, eb * 512:(eb + 1) * 512],
                           start=True, stop=False)
                        mm(yps[:],
                           on[0:64, RB + rb_i * 128:RB + (rb_i + 1) * 128],
                           W2[0:64, dim + eb * 512:dim + (eb + 1) * 512],
                           start=False, stop=True)
                    else:
                        mm(yps[:], on[:, rb_i * 128:(rb_i + 1) * 128],
                           W2[:, eb * 512:(eb + 1) * 512],
                           start=True, stop=True)
                    cpy(ysb[:, eb * 512:(eb + 1) * 512], yps[:])
                    if fine:
                        r_out = base + RB * g + 128 * rb_i
                        nc.sync.dma_start(
                            y[r_out:r_out + 128, eb * 512:(eb + 1) * 512],
                            ysb[:, eb * 512:(eb + 1) * 512])
                if not fine:
                    r_out = base + RB * g + 128 * rb_i
                    nc.sync.dma_start(y[r_out:r_out + 128, :], ysb[:])

        # Schedule: batch-0 scores start as soon as batch-0's projection
        # (sb 0-1) lands, hiding the sb 2-3 x-loads; PV groups interleave
        # with the other batch's scores so PE stays busy while DVE runs the
        # normalize chains.
        if sched in (2, 3):
            # fine weave: batch-1 score windows are pumped one unit at a
            # time between PV / out-proj matmuls, keeping PE dense and ACT
            # continuously fed
            emit_p1(0)
            emit_p1(1)
            emit_scores(0, 0, 6)
            emit_p1(2)
            emit_scores(0, 6, 11)
            emit_p1(3)
            emit_scores(0, 11, jcs)
            pump = make_pump(emit_scores_gen(1, 0, jcs))
            if sched == 3:
                # pump only at points outside any PSUM accumulation chain
                pend = emit_pv_mm(0, 0)
                pump(3)
                for g in range(1, gs):
                    nxt = emit_pv_mm(0, g)
                    pump(2)
                    emit_outproj(0, g - 1, pend, pump=lambda n=1: pump(2 * n))
                    pend = nxt
                emit_outproj(0, gs - 1, pend, pump=lambda n=1: pump(2 * n))
            else:
                pend = emit_pv_mm(0, 0, pump=pump)
                for g in range(1, gs):
                    nxt = emit_pv_mm(0, g, pump=pump)
                    emit_outproj(0, g - 1, pend, pump=pump)
                    pend = nxt
                emit_outproj(0, gs - 1, pend, pump=pump)
            pump(999)  # drain any remaining batch-1 windows
            t_mul = "pool" if tail_mix else "dve"
            if tail_mix == 2:
                t_cps = ["pool", "dve", "pool", "dve", "pool", "dve",
                         "pool", "dve"]
            elif tail_mix:
                t_cps = ["act", "dve", "act", "act", "dve", "act", "act",
                         "dve"]
            else:
                t_cps = None
            pend = emit_pv_mm(1, 0, mulb_eng=t_mul)
            for g in range(1, gs):
                nxt = emit_pv_mm(1, g, mulb_eng=t_mul)
                emit_outproj(1, g - 1, pend, cps=t_cps)
                pend = nxt
            emit_outproj(1, gs - 1, pend, cps=t_cps)
        elif sched == 1:
            emit_p1(0)
            emit_p1(1)
            emit_scores(0, 0, 6)
            emit_p1(2)
            emit_scores(0, 6, 11)
            emit_p1(3)
            emit_scores(0, 11, jcs)
            sc1 = [(0, 2), (2, 7), (7, 12), (12, jcs)]
            if pipe0:
                # pipeline batch-0 PV one group ahead as well: the next
                # group's PV matmuls + the scores slice cover the DVE
                # normalize chain before each out-projection
                pend = emit_pv_mm(0, 0)
                emit_scores(1, *sc1[0])
                for g in range(1, gs):
                    nxt = emit_pv_mm(0, g)
                    emit_outproj(0, g - 1, pend)
                    pend = nxt
                    emit_scores(1, *sc1[g])
                emit_outproj(0, gs - 1, pend)
            else:
                for g in range(gs):
                    pend = emit_pv_mm(0, g)
                    emit_scores(1, *sc1[g])
                    emit_outproj(0, g, pend)
            if tail2:
                # software-pipeline the scores-free batch-1 tail: pv_mm of
                # the next group covers the previous group's DVE normalize
                # chain latency before its out-projection.  tail_mix routes
                # tail copies to the otherwise-idle ACT/Pool engines.
                t_mul = "pool" if tail_mix else "dve"
                if tail_mix == 2:
                    t_cps = ["pool", "dve", "pool", "dve", "pool", "dve",
                             "pool", "dve"]
                elif tail_mix:
                    t_cps = ["act", "dve", "act", "act", "dve", "act", "act",
                             "dve"]
                else:
                    t_cps = None
                pend = emit_pv_mm(1, 0, mulb_eng=t_mul)
                for g in range(1, gs):
                    nxt = emit_pv_mm(1, g, mulb_eng=t_mul)
                    emit_outproj(1, g - 1, pend, cps=t_cps)
                    pend = nxt
                emit_outproj(1, gs - 1, pend, cps=t_cps)
            else:
                for g in range(gs):
                    pend = emit_pv_mm(1, g)
                    emit_outproj(1, g, pend)
        else:
            for sb in range(nsb):
                emit_p1(sb)
            emit_scores(0, 0, jcs)
            emit_scores(1, 0, 4)
            for g in range(gs):
                pend = emit_pv_mm(0, g)
                emit_outproj(0, g, pend)
            emit_scores(1, 4, jcs)
            for g in range(gs):
                pend = emit_pv_mm(1, g)
                emit_outproj(1, g, pend)

        loop_ctx.close()
        if loop_r > 0:
            nc.sync.dma_start(itc[:], itile[:])

    return nc


# ---------------------------------------------------------------------------
# Host wrapper
# ---------------------------------------------------------------------------

_CACHE = {}


def _get_nc():
    if "nc" not in _CACHE:
        import concourse.mybir as mybir
        nc = build_nc()
        _split_waits(nc, mybir, maxw=1)
        _CACHE["nc"] = nc
    return _CACHE["nc"]


def _bf16(a):
    import ml_dtypes
    return np.ascontiguousarray(a.astype(ml_dtypes.bfloat16))


def build_in_maps(x, w_qkv, w_out, osplit=0):
    """Pack host inputs into the per-core DMA-friendly layouts."""
    xf = np.asarray(x, np.float32).reshape(B * SEQ, DIM)
    # xp[p, sb*8192 + k*1024 + c] = xf[sb*1024 + c, k*128 + p]
    xp = xf.reshape(4, 1024, 8, 128).transpose(3, 0, 2, 1).reshape(128, -1)
    xp = _bf16(xp)
    w_qkv = np.asarray(w_qkv, np.float32)
    w_out = np.asarray(w_out, np.float32)
    in_maps = []
    for c in range(N_CORES):
        w1c = w_qkv[:, HD * c:HD * (c + 1)]          # [1024, 128]
        w1p = w1c.reshape(8, 128, 128).transpose(1, 0, 2).reshape(128, 1024)
        w2c = w_out[HD * c:HD * (c + 1), :]          # [128, 1024]
        if osplit:
            # [64, 2*dim]: cols 0:dim = h0 rows, dim:2*dim = h1 rows
            w2c = np.concatenate([w2c[0:64, :], w2c[64:128, :]], axis=1)
        in_maps.append({
            "xp": xp,
            "w1p": _bf16(w1p),
            "w2p": _bf16(w2c),
        })
    return in_maps


def kernel(x, w_qkv, w_out, b_out):
    import jax
    jax.devices()  # ensure axon backend initialized
    from concourse.bass_utils import run_bass_kernel_spmd

    nc = _get_nc()
    in_maps = build_in_maps(x, w_qkv, w_out)
    res = run_bass_kernel_spmd(nc, in_maps, list(range(N_CORES)))
    acc = np.zeros((B * SEQ, DIM), dtype=np.float32)
    for c in range(N_CORES):
        acc += np.asarray(res.results[c]["y"], dtype=np.float32)
    acc += np.asarray(b_out, dtype=np.float32)[None, :]
    return acc.reshape(B, SEQ, DIM)



# revision 62
# speedup vs baseline: 1.6665x; 1.0551x over previous
"""Trainium2 Bass kernel for nn_Attention_15109694947883.

Causal self-attention where (due to the reference's source quirk) q, k, v
all come from the first third of the qkv projection, so only
w_qkv[:, :1024] participates.

Sharding: head-parallel across 8 cores. Core c handles heads (2c, 2c+1)
for both batches: it gets w_qkv columns [128c, 128c+128) and w_out rows
[128c, 128c+128), computes its partial output [4096, 1024]; the host sums
the 8 partials and adds the bias.

v4 changes (TimelineSim 157us -> 131.8us single-core; HW per-iter
243us -> ~235us and below, R=16385 For_i amplification):
  - projection PSUM moved off the scores "sp" tag onto "pa" (the big
    one: -18us sim — proj and scores no longer serialize on the same
    PSUM rotation).
  - exp activations at act_w=1024 (two PSUM banks per exp): fewer,
    wider ACT instructions; out-proj yps shares the freed "pa" tag.
  - merged QNX copy: one strided DVE copy per transpose block
    ([2x64-col groups at stride 65]) instead of two.
  - normalize: mulB emitted before the partition-shift DMA so the
    shift overlaps mulA; optional Pool-engine shift (shift_eng).
  - final out-proj group stores y per 512-col half (ysplit) to pull
    the last DMA earlier; warm0 pre-warms PE on the identity before
    W1 lands.
  - sched=2/3: batch-1 score windows woven one unit at a time between
    batch-0 PV/out-proj matmuls (PE density for the HW HAM clock).
  - NOTE hard-learned HW constraints: Pool (gpsimd) cannot read PSUM
    (compile error); matmul out must be fp32 PSUM at base partition
    0/32/64; stream_shuffle is quadrant-local.

v3 design (vs the 341us baseline; measured 243us via R=16385 For_i
amplification, TimelineSim 157us single-core):
  - bf16 everywhere on the matmul path (x, w1, w2, Q, scores-P, y) --
    halves HBM traffic; bf16 transposes run 1 cycle/row.
  - host-packed input layouts so the x load is 4 large fully-contiguous
    DMAs (16KB/descriptor) instead of 64 small ones: each DMA costs
    ~625ns of serialized HWDGE dispatch regardless of size.
  - softmax denominator folded into the PV matmul: lhsT is the 65-wide
    [V-dims | ones] slice of QNX, so out rows 0:64 = PV and row 64 = the
    denominator. Kills 160 separate denominator matmuls (~29us of PE).
  - causal diag-block masking moved off DVE: exp runs unmasked, then the
    idle Pool engine zeroes the upper-triangular part of the diagonal
    P^T block (affine_select on bf16 in SBUF).
  - exp activations widened to 1024 columns (two PSUM banks per
    activation) to halve ACT instruction overhead.
  - y written as [128,1024] bf16 rows: 32 stores of 2KB/partition.
  - emission schedule (sched=1): batch-0 scores start after half the
    projection (hides the x-load DMA), and each PV group is followed by
    a slice of the other batch's scores so the PE stays busy while DVE
    runs the normalize chain. Sim slightly prefers sched=0, but HW is
    ~29us faster with sched=1 (PE density keeps the HAM clock warm).
  - PE warmup burst during the first x-load DMA (HAM clock-gate stays
    at 2.4GHz), head-interleaved score windows (adjacent matmuls on
    different PE row tiles), and a software-pipelined batch-1 tail
    (pv_mm of group g+1 covers group g's DVE normalize-chain latency).
"""

import numpy as np

# Problem constants (hardcoded per harness contract)
B = 2
SEQ = 2048
DIM = 1024
HEADS = 16
DH = 64
SCALE = DH ** -0.5
N_CORES = 8
HD = 128          # head dims per core = 2 heads x 64
CB = 128          # key block
RB = 512          # row group block


def _split_waits(nc, mybir, maxw=1):
    """This walrus build rejects >maxw sync waits on one instruction
    (seen on Tile's tail drain). Split excess waits onto preceding
    same-engine NoOps — engines execute their stream in order, so the
    blocking semantics are identical."""
    n = 0
    for f in nc.m.functions:
        for bb in f.blocks:
            insts = list(bb.instructions)
            out = []
            for inst in insts:
                si = inst.sync_info
                if si is not None and si.on_wait and len(si.on_wait) > maxw:
                    waits = list(si.on_wait)
                    head, rest = waits[:-maxw], waits[-maxw:]
                    while head:
                        chunk, head = head[:maxw], head[maxw:]
                        nop = mybir.InstNoOp(
                            name=f"I-waitsplit-{nc.next_id()}", ins=[], outs=[]
                        )
                        nop.engine = inst.engine
                        nop.sync_info = mybir.SyncInfo(
                            on_wait=chunk, on_update=[]
                        )
                        out.append(nop)
                        n += 1
                    si.on_wait = rest
                out.append(inst)
            if len(out) != len(insts):
                bb.instructions = out
    return n


def build_nc(seq=SEQ, dim=DIM, b=B, loop_r=0, act_w=1024, sp_bufs=2,
             yps_tag="pa", yps_bufs=2, ab_bufs=2, null_body=False, sched=2,
             hil=1, warm=10, tail2=1, pipe0=1, ysb_eng="dve", qnx_merge=1,
             yps_bf16=0, osplit=0, aff_early=0, qps_tag="pa", bcast=0,
             shift_eng="dve", tail_mix=1, xsplit=0, warm0=0, ysplit=1,
             sr=0, tail_yt=1):
    # Measured 218998 ns/iter (R=65537, sr=1, min of 9 interleaved
    # rounds in a clean machine window; the previous defaults measured
    # 231901 in the same window).  TimelineSim 127.1us.  The v5 tail
    # bundle: tail out-proj PSUM on the freed "sp" tag (tail_yt), tail
    # ysb copies on the otherwise-idle ACT engine (tail_mix=1), the
    # h1 partition shift as a DVE offset copy instead of a DMA
    # (shift_eng="dve" — verified correct on HW), and split final y
    # stores (ysplit).
    from contextlib import ExitStack

    import concourse.bass as bass
    import concourse.mybir as mybir
    import concourse.tile as tile
    from concourse.masks import make_identity

    f32 = mybir.dt.float32
    bf16 = mybir.dt.bfloat16

    nb = b * seq              # 4096 total rows
    kt = dim // 128           # 8 contraction tiles
    nsb = nb // 1024          # 4 projection super-blocks
    nblk = nb // 128          # 32 transpose blocks
    jcs = seq // CB           # 16 key blocks per batch
    gs = seq // RB            # 4 row groups per batch
    QB = 130                  # QNX cols per 128-row block: [h0 64|1|h1 64|1]

    nc = bass.Bass("TRN2", target_bir_lowering=False, debug=False)
    # host-packed x: xp[p, sb*8192 + k*1024 + c] = x[sb*1024+c, k*128+p]
    xp = nc.dram_tensor("xp", [128, kt * nb], bf16, kind="ExternalInput").ap()
    # host-packed w1: w1p[p, k*128 + j] = w_qkv[k*128+p, 128*core + j]
    w1p = nc.dram_tensor("w1p", [128, dim], bf16, kind="ExternalInput").ap()
    # osplit: w2p[p, h*dim + j] = w_out[HD*core + 64*h + p, j]  ([64, 2*dim])
    w2p = (nc.dram_tensor("w2p", [64, 2 * dim], bf16, kind="ExternalInput").ap()
           if osplit else
           nc.dram_tensor("w2p", [HD, dim], bf16, kind="ExternalInput").ap())
    y = nc.dram_tensor("y", [nb, dim], bf16, kind="ExternalOutput").ap()
    itc = (nc.dram_tensor("itc", [1, 1], f32, kind="ExternalOutput").ap()
           if loop_r > 0 else None)

    mm = nc.tensor.matmul

    with tile.TileContext(nc) as tc, ExitStack() as ctx:
        cpool = ctx.enter_context(tc.tile_pool(name="consts", bufs=1))
        ident = cpool.tile([128, 128], bf16, tag="ident")
        make_identity(nc, ident[:])

        wpool = ctx.enter_context(tc.tile_pool(name="w", bufs=1))
        W1 = wpool.tile([128, dim], bf16, tag="w1")
        nc.sync.dma_start(W1[:], w1p[:, :])
        if osplit:
            W2 = wpool.tile([64, 2 * dim], bf16, tag="w2")
        else:
            W2 = wpool.tile([128, dim], bf16, tag="w2")
        nc.sync.dma_start(W2[:], w2p[:, :])

        qpool = ctx.enter_context(tc.tile_pool(name="q", bufs=1))
        QT = qpool.tile([128, nb], bf16, tag="qt")       # [head-dim, row]
        QNX = qpool.tile([128, nblk * QB], bf16, tag="qnx")

        psum = ctx.enter_context(tc.tile_pool(name="ps", bufs=1, space="PSUM"))
        ptpool = ctx.enter_context(tc.tile_pool(name="pt", bufs=1))
        spool = ctx.enter_context(tc.tile_pool(name="sm", bufs=2))
        onpool = ctx.enter_context(tc.tile_pool(name="on", bufs=2))
        ypool = ctx.enter_context(tc.tile_pool(name="ysb",
                                               bufs=2 if osplit else 3))
        xpool = ctx.enter_context(tc.tile_pool(name="xt", bufs=2))

        if loop_r > 0:
            itile = cpool.tile([1, 1], f32, tag="itile")
            nc.gpsimd.memset(itile[:], 0.0)

        loop_ctx = ExitStack()
        if loop_r > 0:
            loop_ctx.enter_context(
                tc.For_i(0, loop_r, 1, staggered_reset=bool(sr)))
            it2 = cpool.tile([1, 1], f32, tag="it2")
            nc.scalar.add(it2[:], itile[:], 1.0)
            nc.vector.tensor_copy(itile[:], it2[:])

        if null_body:
            # minimal loop body: one tiny matmul + one y-store, to measure
            # the fixed per-iteration For_i overhead.
            # null_body=2: counter only (no mm/copy/DMA); 3: +mm+copy, no DMA
            if null_body != 2:
                nps = psum.tile([128, 128], f32, tag="sp", bufs=sp_bufs,
                                padded_shape=[128, act_w])
                mm(nps[:], W1[:, 0:128], W1[:, 0:128],
                   start=True, stop=True)
                nsb_t = ypool.tile([128, 128], bf16, tag="ysb",
                                   padded_shape=[128, dim])
                nc.vector.tensor_copy(nsb_t[:], nps[:])
                if null_body != 3:
                    nc.sync.dma_start(y[0:128, 0:128], nsb_t[:])
            loop_ctx.close()
            if loop_r > 0:
                nc.sync.dma_start(itc[:], itile[:])
            return nc

        # ---- Phase 1: projection QT = w1^T x^T, transposes into QNX ----
        # QNX per block: [h0 dims 0:64 | ones | h1 dims 0:64 | ones];
        # memset(1.0) first, copies overwrite the data columns.
        nc.gpsimd.memset(QNX[:], 1.0)
        if warm0:
            # pre-warm on the engine-generated identity (no DMA dependency):
            # keeps the PE HAM clock ramping before W1 even lands
            w0ps = psum.tile([128, 128], f32, tag="pa", bufs=ab_bufs,
                             padded_shape=[128, 512])
            for i in range(warm0):
                mm(w0ps[:], ident[:], ident[:],
                   start=(i == 0), stop=(i == warm0 - 1),
                   skip_group_check=True)
        if warm:
            # keep the PE HAM clock-gate warm while the first x tiles stream
            # in: dummy accumulating matmuls on already-resident weights
            wps = psum.tile([128, RB], f32, tag="pa", bufs=ab_bufs)
            wrhs = W1 if osplit else W2
            for i in range(warm):
                mm(wps[0:128, :], W1[:, (i % 2) * 128:(i % 2) * 128 + 128],
                   wrhs[:, 0:RB], start=(i == 0), stop=(i == warm - 1),
                   skip_group_check=True)

        def emit_p1(sb):
            xsb = xpool.tile([128, kt * 1024], bf16, tag="xsb", bufs=2)
            if xsplit:
                # two DMAs: c-halves of every k-tile, so the half-0 matmuls
                # can start as soon as the first (smaller) DMA lands
                xv = xsb[:].rearrange("p (k c) -> p k c", k=kt)
                sv = xp[:, sb * kt * 1024:(sb + 1) * kt * 1024].rearrange(
                    "p (k c) -> p k c", k=kt)
                nc.sync.dma_start(xv[:, :, 0:512], sv[:, :, 0:512])
                nc.sync.dma_start(xv[:, :, 512:1024], sv[:, :, 512:1024])
            else:
                nc.sync.dma_start(
                    xsb[:], xp[:, sb * kt * 1024:(sb + 1) * kt * 1024])
            qhs = []
            for half in range(2):
                qps = psum.tile([128, 512], f32,
                                tag=qps_tag,
                                bufs=(sp_bufs if qps_tag == "sp" else ab_bufs),
                                padded_shape=[128, max(512, act_w)]
                                if qps_tag == "sp" else None)
                qhs.append(qps)
                for k in range(kt):
                    mm(qps[:, 0:512],
                       W1[:, k * 128:(k + 1) * 128],
                       xsb[:, k * 1024 + half * 512:k * 1024 + (half + 1) * 512],
                       start=(k == 0), stop=(k == kt - 1),
                       skip_group_check=True)
            for half in range(2):
                nc.vector.tensor_copy(
                    QT[:, sb * 1024 + half * 512:sb * 1024 + (half + 1) * 512],
                    qhs[half][:, 0:512])
            for t in range(8):
                col = sb * 1024 + t * 128
                blk = col // 128
                tps = psum.tile([128, 128], bf16, tag="pb", bufs=ab_bufs,
                                padded_shape=[128, 2 * RB])
                nc.tensor.transpose(tps[:], QT[:, col:col + 128], ident[:])
                if qnx_merge:
                    # single strided copy: 2 groups of 64 cols, dst stride 65
                    dst = QNX[:, blk * QB:blk * QB + 130].rearrange(
                        "p (two c) -> p two c", two=2)[:, :, 0:64]
                    src = tps[:].rearrange("p (two c) -> p two c", two=2)
                    nc.vector.tensor_copy(dst, src)
                else:
                    nc.vector.tensor_copy(QNX[:, blk * QB:blk * QB + 64],
                                          tps[:, 0:64])
                    nc.vector.tensor_copy(QNX[:, blk * QB + 65:blk * QB + 129],
                                          tps[:, 64:128])

        # ---- Phase 2: attention ----
        PTs = [dict() for _ in range(b)]

        def emit_scores_gen(bi, jc_lo, jc_hi):
            """Generator form: yields after each (h, t) window unit so the
            caller can weave score emission between other PE work."""
            base = bi * seq
            PT = PTs[bi]
            for jc in range(jc_lo, jc_hi):
                r0 = CB * jc
                cw = seq - r0
                for h in range(2):
                    pt = ptpool.tile([128, cw], bf16, tag=f"pt{h}_{jc}",
                                     bufs=2)
                    PT[(h, jc)] = pt
                if hil:
                    order = [(h, t) for t in range(r0 // act_w, seq // act_w)
                             for h in range(2)]
                else:
                    order = [(h, t) for h in range(2)
                             for t in range(r0 // act_w, seq // act_w)]
                def emit_mask(h):
                    # zero the strictly-upper part of the diagonal block
                    # (query col rr < key row c) on the idle Pool engine
                    pt = PT[(h, jc)]
                    nc.gpsimd.affine_select(
                        out=pt[:, 0:128], in_=pt[:, 0:128],
                        compare_op=mybir.AluOpType.is_ge, fill=0.0,
                        base=0, pattern=[[1, 128]], channel_multiplier=-1,
                    )
                diag_t = r0 // act_w
                for h, t in order:
                    pt = PT[(h, jc)]
                    lhsT = QT[64 * h:64 * h + 64, base + r0:base + r0 + 128]
                    ws = max(act_w * t, r0)
                    we = act_w * (t + 1)
                    sp = psum.tile([128, act_w], f32, tag="sp",
                                   bufs=sp_bufs, padded_shape=[128, act_w])
                    cs = ws
                    while cs < we:
                        ce = min(we, (cs // 512 + 1) * 512)
                        mm(sp[:, cs - act_w * t:ce - act_w * t], lhsT,
                           QT[64 * h:64 * h + 64, base + cs:base + ce],
                           start=True, stop=True,
                           tile_position=(64 * h, 0))
                        cs = ce
                    nc.scalar.activation(
                        pt[:, ws - r0:we - r0],
                        sp[:, ws - act_w * t:we - act_w * t],
                        mybir.ActivationFunctionType.Exp,
                        bias=0.0, scale=float(SCALE))
                    if aff_early and t == diag_t:
                        emit_mask(h)
                    yield
                if not aff_early:
                    for h in range(2):
                        emit_mask(h)

        def emit_scores(bi, jc_lo, jc_hi):
            for _ in emit_scores_gen(bi, jc_lo, jc_hi):
                pass

        def make_pump(gen):
            def pump(n=1):
                for _ in range(n):
                    if next(gen, "end") == "end":
                        break
            return pump

        ENG_CPY = {"dve": nc.vector.tensor_copy,
                   "pool": nc.gpsimd.tensor_copy,
                   "act": nc.scalar.copy}
        ENG_MUL = {"dve": nc.vector.tensor_mul,
                   "pool": nc.gpsimd.tensor_mul}

        def emit_pv_mm(bi, g, mulb_eng="dve", pump=None):
            """PV+denominator matmuls for row group g; returns the 'on' tile
            (normalized O^T) whose out-projection the caller emits later."""
            base = bi * seq
            PT = PTs[bi]
            A = psum.tile([128, RB], f32, tag="pa", bufs=ab_bufs)
            Bp = psum.tile([128, RB], f32, tag="pb", bufs=ab_bufs,
                           padded_shape=[128, RB])
            njc = (g + 1) * (RB // CB)
            for jc in range(njc):
                r0 = CB * jc
                cs = max(RB * g, r0)
                w = RB * (g + 1) - cs
                blk = bi * jcs + jc
                for h, T in ((0, A), (1, Bp)):
                    pts = PT[(h, jc)][:, cs - r0:cs - r0 + w]
                    mm(T[0:65, cs - RB * g:cs - RB * g + w],
                       QNX[:, blk * QB + 65 * h:blk * QB + 65 * h + 65],
                       pts,
                       start=(jc == 0), stop=(jc == njc - 1),
                       skip_group_check=True)
                if pump is not None and jc % 2 == 1:
                    pump(1)
            # reciprocal of the folded denominators (partition 64)
            rr = spool.tile([128, 1024], bf16, tag="rr")
            with nc.allow_low_precision(reason="1/den in bf16: 0.4% rel"):
                nc.vector.reciprocal(rr[64:65, 0:RB], A[64:65, :])
                nc.vector.reciprocal(rr[64:65, RB:2 * RB], Bp[64:65, :])
            # broadcast partition 64 -> partitions 0:64 (h0 cols 0:512,
            # h1 cols 512:1024)
            bc = spool.tile([128, 1024], bf16, tag="bc")
            if bcast:
                nc.gpsimd.partition_broadcast(bc[0:64, :], rr[64:65, :])
            else:
                nc.sync.dma_start(
                    bc[0:64, :],
                    rr[64:65, :].unsqueeze(1).to_broadcast([1, 64, 1024]))
            if osplit:
                # keep both heads at partitions 0:64 ([64, 2*RB]: h0 cols
                # 0:RB, h1 cols RB:2RB); out-proj contracts each head
                # separately (no partition-shift DMA needed)
                on = onpool.tile([64, 2 * RB], bf16, tag="on")
                nc.vector.tensor_mul(on[0:64, 0:RB], A[0:64, :],
                                     bc[0:64, 0:RB])
                nc.vector.tensor_mul(on[0:64, RB:2 * RB], Bp[0:64, :],
                                     bc[0:64, RB:2 * RB])
                return on
            on = onpool.tile([128, RB], bf16, tag="on")
            tmp = onpool.tile([128, RB], bf16, tag="tmp")
            # mulB first so the partition shift overlaps mulA on DVE
            ENG_MUL[mulb_eng](tmp[0:64, :], Bp[0:64, :],
                              bc[0:64, RB:2 * RB])
            # partition shift h1 dims into on[64:128]
            if shift_eng == "dve":
                nc.vector.tensor_copy(on[64:128, :], tmp[0:64, :])
            elif shift_eng == "pool":
                nc.gpsimd.tensor_copy(on[64:128, :], tmp[0:64, :])
            else:
                nc.sync.dma_start(on[64:128, :], tmp[0:64, :])
            nc.vector.tensor_mul(on[0:64, :], A[0:64, :], bc[0:64, 0:RB])
            return on

        def emit_outproj(bi, g, on, cps=None, pump=None, ytag=None):
            base = bi * seq
            if cps is None:
                cps = [ysb_eng] * 8
            fine = ysplit and bi == b - 1 and g == gs - 1
            for rb_i in range(RB // 128):
                if pump is not None:
                    pump(1)
                ysb = ypool.tile([128, dim], bf16, tag="ysb")
                for eb in range(2):
                    cpy = ENG_CPY[cps[rb_i * 2 + eb]]
                    ytag_ = ytag or yps_tag
                    yps = psum.tile([128, 512], bf16 if yps_bf16 else f32,
                                    tag=ytag_,
                                    bufs=sp_bufs if ytag_ == "sp" else yps_bufs,
                                    padded_shape=([128, act_w]
                                                  if ytag_ == "sp" else None))
                    if osplit:
                        mm(yps[:], on[0:64, rb_i * 128:(rb_i + 1) * 128],
                           W2[0:64, eb * 512:(eb + 1) * 512],
                           start=True, stop=False)
                        mm(yps[:],
                           on[0:64, RB + rb_i * 128:RB + (rb_i + 1) * 128],
                           W2[0:64, dim + eb * 512:dim + (eb + 1) * 512],
                           start=False, stop=True)
                    else:
                        mm(yps[:], on[:, rb_i * 128:(rb_i + 1) * 128],
                           W2[:, eb * 512:(eb + 1) * 512],
                           start=True, stop=True)
                    cpy(ysb[:, eb * 512:(eb + 1) * 512], yps[:])
                    if fine:
                        r_out = base + RB * g + 128 * rb_i
                        nc.sync.dma_start(
                            y[r_out:r_out + 128, eb * 512:(eb + 1) * 512],
                            ysb[:, eb * 512:(eb + 1) * 512])
                if not fine:
                    r_out = base + RB * g + 128 * rb_i
                    nc.sync.dma_start(y[r_out:r_out + 128, :], ysb[:])

        # Schedule: batch-0 scores start as soon as batch-0's projection
        # (sb 0-1) lands, hiding the sb 2-3 x-loads; PV groups interleave
        # with the other batch's scores so PE stays busy while DVE runs the
        # normalize chains.
        if sched in (2, 3):
            # fine weave: batch-1 score windows are pumped one unit at a
            # time between PV / out-proj matmuls, keeping PE dense and ACT
            # continuously fed
            emit_p1(0)
            emit_p1(1)
            emit_scores(0, 0, 6)
            emit_p1(2)
            emit_scores(0, 6, 11)
            emit_p1(3)
            emit_scores(0, 11, jcs)
            pump = make_pump(emit_scores_gen(1, 0, jcs))
            if sched == 3:
                # pump only at points outside any PSUM accumulation chain
                pend = emit_pv_mm(0, 0)
                pump(3)
                for g in range(1, gs):
                    nxt = emit_pv_mm(0, g)
                    pump(2)
                    emit_outproj(0, g - 1, pend, pump=lambda n=1: pump(2 * n))
                    pend = nxt
                emit_outproj(0, gs - 1, pend, pump=lambda n=1: pump(2 * n))
            else:
                pend = emit_pv_mm(0, 0, pump=pump)
                for g in range(1, gs):
                    nxt = emit_pv_mm(0, g, pump=pump)
                    emit_outproj(0, g - 1, pend, pump=pump)
                    pend = nxt
                emit_outproj(0, gs - 1, pend, pump=pump)
            pump(999)  # drain any remaining batch-1 windows
            # NOTE: Pool cannot read PSUM on this HW (compile error), so
            # the normalize muls must stay on DVE
            t_mul = "dve"
            if tail_mix == 2:
                t_cps = ["pool", "dve", "pool", "dve", "pool", "dve",
                         "pool", "dve"]
            elif tail_mix:
                t_cps = ["act", "dve", "act", "act", "dve", "act", "act",
                         "dve"]
            else:
                t_cps = None
            t_yt = "sp" if tail_yt else None
            pend = emit_pv_mm(1, 0, mulb_eng=t_mul)
            for g in range(1, gs):
                nxt = emit_pv_mm(1, g, mulb_eng=t_mul)
                emit_outproj(1, g - 1, pend, cps=t_cps, ytag=t_yt)
                pend = nxt
            emit_outproj(1, gs - 1, pend, cps=t_cps, ytag=t_yt)
        elif sched == 1:
            emit_p1(0)
            emit_p1(1)
            emit_scores(0, 0, 6)
            emit_p1(2)
            emit_scores(0, 6, 11)
            emit_p1(3)
            emit_scores(0, 11, jcs)
            sc1 = [(0, 2), (2, 7), (7, 12), (12, jcs)]
            if pipe0:
                # pipeline batch-0 PV one group ahead as well: the next
                # group's PV matmuls + the scores slice cover the DVE
                # normalize chain before each out-projection
                pend = emit_pv_mm(0, 0)
                emit_scores(1, *sc1[0])
                for g in range(1, gs):
                    nxt = emit_pv_mm(0, g)
                    emit_outproj(0, g - 1, pend)
                    pend = nxt
                    emit_scores(1, *sc1[g])
                emit_outproj(0, gs - 1, pend)
            else:
                for g in range(gs):
                    pend = emit_pv_mm(0, g)
                    emit_scores(1, *sc1[g])
                    emit_outproj(0, g, pend)
            if tail2:
                # software-pipeline the scores-free batch-1 tail: pv_mm of
                # the next group covers the previous group's DVE normalize
                # chain latency before its out-projection.  tail_mix routes
                # tail copies to the otherwise-idle ACT/Pool engines.
                t_mul = "pool" if tail_mix else "dve"
                if tail_mix == 2:
                    t_cps = ["pool", "dve", "pool", "dve", "pool", "dve",
                             "pool", "dve"]
                elif tail_mix:
                    t_cps = ["act", "dve", "act", "act", "dve", "act", "act",
                             "dve"]
                else:
                    t_cps = None
                pend = emit_pv_mm(1, 0, mulb_eng=t_mul)
                for g in range(1, gs):
                    nxt = emit_pv_mm(1, g, mulb_eng=t_mul)
                    emit_outproj(1, g - 1, pend, cps=t_cps)
                    pend = nxt
                emit_outproj(1, gs - 1, pend, cps=t_cps)
            else:
                for g in range(gs):
                    pend = emit_pv_mm(1, g)
                    emit_outproj(1, g, pend)
        else:
            for sb in range(nsb):
                emit_p1(sb)
            emit_scores(0, 0, jcs)
            emit_scores(1, 0, 4)
            for g in range(gs):
                pend = emit_pv_mm(0, g)
                emit_outproj(0, g, pend)
            emit_scores(1, 4, jcs)
            for g in range(gs):
                pend = emit_pv_mm(1, g)
                emit_outproj(1, g, pend)

        loop_ctx.close()
        if loop_r > 0:
            nc.sync.dma_start(itc[:], itile[:])

    return nc


# ---------------------------------------------------------------------------
# Host wrapper
# ---------------------------------------------------------------------------

_CACHE = {}


def _get_nc():
    if "nc" not in _CACHE:
        import concourse.mybir as mybir
        nc = build_nc()
        _split_waits(nc, mybir, maxw=1)
        _CACHE["nc"] = nc
    return _CACHE["nc"]


def _bf16(a):
    import ml_dtypes
    return np.ascontiguousarray(a.astype(ml_dtypes.bfloat16))


def build_in_maps(x, w_qkv, w_out, osplit=0):
    """Pack host inputs into the per-core DMA-friendly layouts."""
    xf = np.asarray(x, np.float32).reshape(B * SEQ, DIM)
    # xp[p, sb*8192 + k*1024 + c] = xf[sb*1024 + c, k*128 + p]
    xp = xf.reshape(4, 1024, 8, 128).transpose(3, 0, 2, 1).reshape(128, -1)
    xp = _bf16(xp)
    w_qkv = np.asarray(w_qkv, np.float32)
    w_out = np.asarray(w_out, np.float32)
    in_maps = []
    for c in range(N_CORES):
        w1c = w_qkv[:, HD * c:HD * (c + 1)]          # [1024, 128]
        w1p = w1c.reshape(8, 128, 128).transpose(1, 0, 2).reshape(128, 1024)
        w2c = w_out[HD * c:HD * (c + 1), :]          # [128, 1024]
        if osplit:
            # [64, 2*dim]: cols 0:dim = h0 rows, dim:2*dim = h1 rows
            w2c = np.concatenate([w2c[0:64, :], w2c[64:128, :]], axis=1)
        in_maps.append({
            "xp": xp,
            "w1p": _bf16(w1p),
            "w2p": _bf16(w2c),
        })
    return in_maps


def kernel(x, w_qkv, w_out, b_out):
    import jax
    jax.devices()  # ensure axon backend initialized
    from concourse.bass_utils import run_bass_kernel_spmd

    nc = _get_nc()
    in_maps = build_in_maps(x, w_qkv, w_out)
    res = run_bass_kernel_spmd(nc, in_maps, list(range(N_CORES)))
    acc = np.zeros((B * SEQ, DIM), dtype=np.float32)
    for c in range(N_CORES):
        acc += np.asarray(res.results[c]["y"], dtype=np.float32)
    acc += np.asarray(b_out, dtype=np.float32)[None, :]
    return acc.reshape(B, SEQ, DIM)

